# revision 20
# baseline (speedup 1.0000x reference)
"""GAT (2-layer, 4-head) Trainium2 Bass kernel, 8-core SPMD — v4.

Layer 1: host lays out x[src] in (window, slot, lane) cell order; device
streams it and computes h1 + a_s1 per cell on PE (attention vectors folded
into extra matmul columns). Pad cells hold a vector v with v@wsrc1_h = -160
for every head, so pad logits underflow exp to exact 0 — no mask tensor.
Layer 2: dst-major dma_gather (split across 4 SWDGE queues — the gather is
descriptor-rate-bound per queue) from the AllGather'd T2 row table; pad
cells point at a dedicated pad row storing a_s2 = -160. Rows store
elu(h)+1; the -1 is folded into per-head logit constants and the output
bias (softmax weights sum to 1, so the shift is exact).
Softmax + weighted aggregation run dst-major on DVE; PSUM->SBUF cell
copies alternate between ACT and Pool to balance engine load.
"""

import os
import numpy as np
from contextlib import ExitStack

import concourse.bass as bass
import concourse.tile as tile
from concourse import bacc, mybir
from concourse.bass_utils import run_bass_kernel_spmd

# problem constants (hardcoded per contest contract)
N = 50000
E = 1600000
HEADS = 4
HID = 32
INF = 128
OUTF = 8
NCORES = 8
NLOC = N // NCORES            # 6250 dst per core
WPC = (NLOC + 127) // 128     # 49 windows per core
NPAD = WPC * 128              # 6272
TB2 = NCORES * NPAD           # 50176 rows in layer-2 table
BASE = 17408                  # mid-base for signed int16 gather indices
PADROW = 2 * NPAD + NLOC      # a pad-lane row (zero h, a_s2=-160); idx>=0
ROWW = 256                    # fp16 words per T2 row (512 B)
L1W = 132                     # fp16 words per L1 cell row (h 128 + a_s 4)
DCAP = 32                     # slot-chunk for multiply/reduce working tile
PIECE = 16                    # stream chunks (of 128 cells) per DMA piece
NQ = 4                        # SWDGE queues; L2 window gathers split across
NEGC = -160.0                 # pad logit level (leaky*0.2 -> -32, exp -> 0)

F32 = mybir.dt.float32
F16 = mybir.dt.float16
I16 = mybir.dt.int16

_CACHE = {}
LAST_RESULT = None
LAST_NC = None
LAST_IN_MAPS = None


def _qsegs(d):
    """Split d slots into NQ contiguous per-queue segments (some may be empty)."""
    dq = (d + NQ - 1) // NQ
    segs = []
    for q in range(NQ):
        q0, q1 = q * dq, min(d, (q + 1) * dq)
        if q1 > q0:
            segs.append((q0, q1))
    return segs


# ----------------------------------------------------------------------------
# host-side graph preprocessing
# ----------------------------------------------------------------------------

def _host_prep(edge_index):
    srcs = np.concatenate([edge_index[0], np.arange(N)]).astype(np.int64)
    dsts = np.concatenate([edge_index[1], np.arange(N)]).astype(np.int64)
    ne = srcs.shape[0]

    core = dsts // NLOC
    deg = np.bincount(dsts, minlength=N)

    perms = []
    pos = np.empty(N, np.int64)
    for c in range(NCORES):
        p = np.argsort(-deg[c * NLOC:(c + 1) * NLOC], kind="stable")
        perms.append(p)
        pos[c * NLOC + p] = np.arange(NLOC)

    wpos = pos[dsts]
    w_e = wpos // 128
    lane_e = wpos % 128

    # slot j within each (core, dst) lane, in edge order
    key = core * NLOC + wpos
    order = np.argsort(key, kind="stable")
    ks = key[order]
    change = np.r_[True, ks[1:] != ks[:-1]]
    startpos = np.flatnonzero(change)
    gid = np.cumsum(change) - 1
    j_sorted = np.arange(ne) - startpos[gid]
    j = np.empty(ne, np.int64)
    j[order] = j_sorted

    degs = np.zeros((NCORES, NPAD), np.int64)
    for c in range(NCORES):
        degs[c, :NLOC] = deg[c * NLOC + perms[c]]
    d_w = degs.reshape(NCORES, WPC, 128).max(axis=(0, 2))
    d_w[WPC - 1] = max(int(d_w[WPC - 1]), 1)
    cs = np.r_[0, np.cumsum(d_w)]
    tot = int(cs[-1])
    dmax = int(d_w.max())
    assert dmax <= 80, dmax

    cellpos = (cs[w_e] + j) * 128 + lane_e  # flat cell column per edge

    src_cell = np.full((NCORES, tot * 128), -1, np.int64)
    src_cell[core, cellpos] = srcs

    t2row = (np.arange(N) // NLOC) * NPAD + pos

    idxv = np.full((NCORES, tot * 128), PADROW - BASE, np.int16)
    real = src_cell >= 0
    idxv[real] = (t2row[src_cell[real]] - BASE).astype(np.int16)

    # Trim-safety: Q7 ucode drops trailing NEGATIVE indices of a gather, so
    # the LAST index of every per-queue gather segment must be >= 0 or real
    # cells would be silently dropped. Pads are PADROW-BASE > 0 (safe); for
    # a real negative cell swap slots within lane 127 (slot order within a
    # lane is irrelevant) to put a nonneg-index cell at each segment tail.
    for c in range(NCORES):
        for w in range(WPC):
            cw = int(cs[w])
            d = int(d_w[w])
            tails = {q1 - 1 for _, q1 in _qsegs(d)}
            for tj in sorted(tails):
                lastc = (cw + tj) * 128 + 127
                if idxv[c, lastc] >= 0:
                    continue
                fixed = False
                for jj in range(d):
                    if jj in tails:
                        continue
                    col = (cw + jj) * 128 + 127
                    if idxv[c, col] < 0:
                        continue
                    for arr in (idxv, src_cell):
                        arr[c, col], arr[c, lastc] = arr[c, lastc], arr[c, col]
                    fixed = True
                    break
                assert fixed, f"unfixable trim boundary core {c} window {w}"

    # wrap idx into dma_gather layout [128, 8*tot] (16-partition wrap, 8x rep)
    idxw = np.zeros((NCORES, 128, 8 * tot), np.int16)
    for w in range(WPC):
        d = int(d_w[w])
        cw = int(cs[w])
        blk = idxv[:, cw * 128:(cw + d) * 128]                 # [NC, d*128]
        blk = blk.reshape(NCORES, -1, 16).transpose(0, 2, 1)   # [NC, 16, 8d]
        idxw[:, :, 8 * cw: 8 * (cw + d)] = np.tile(blk, (1, 8, 1))

    return dict(d_w=d_w, cs=cs, tot=tot, dmax=dmax,
                src_cell=src_cell, idxw=idxw), perms


def _fold_weights(W1, att_src1, att_dst1, b1, W2, att_src2, att_dst2, b2, Wout, bout):
    # device feature order is head-interleaved: dev k = c*4 + h <-> ref h*32 + c
    perm = np.array([h * 32 + c for c in range(HID) for h in range(HEADS)])

    def vec(att):  # [HEADS, HID] -> [128, 4] fold in dev space
        z = np.zeros((INF, HEADS), np.float32)
        k = np.arange(INF)
        z[k, k % HEADS] = att[k % HEADS, k // HEADS]
        return z

    W1d = W1[:, perm].astype(np.float64)
    wsx1 = W1d @ vec(att_src1).astype(np.float64)                  # [128,4]
    rhs1 = np.concatenate([W1d, wsx1], axis=1).astype(np.float16)  # [128,132]
    wdx1 = (W1d @ vec(att_dst1).astype(np.float64)).astype(np.float16)

    # pad-cell vector: v @ wsx1_h = NEGC for every head (min-norm solution)
    v = (wsx1 @ np.linalg.solve(wsx1.T @ wsx1, np.full(HEADS, NEGC)))
    v16 = v.astype(np.float16)

    W2d = W2[perm][:, perm].astype(np.float64)
    ws2 = W2d @ vec(att_src2).astype(np.float64)
    wd2 = W2d @ vec(att_dst2).astype(np.float64)
    rhs2 = np.concatenate([W2d, ws2, wd2], axis=1).astype(np.float32)  # [128,136]
    woutd = Wout[perm].astype(np.float64)                               # [128,8]

    # elu+1 fold: table rows store y = elu(t)+1, so ps2 = true + colsum consts
    c2 = W2d.sum(axis=0)                          # [128] h2 shift
    cs2 = ws2.sum(axis=0)                         # [4] a_s2 shift
    cd2 = wd2.sum(axis=0)                         # [4] a_d2 shift
    cstm = np.tile((-(cs2 + cd2)).astype(np.float32), (128, 1))        # [128,4]

    b1t = np.tile(b1[perm].astype(np.float32), (128, 1))               # [128,128]
    bf = (b2 @ Wout + bout) - c2 @ woutd          # fold AGG2 shift into bias
    bft = np.tile(bf.astype(np.float32), (128, 1))                     # [128,8]
    return (rhs1, wdx1, rhs2, woutd.astype(np.float32), b1t, bft, cstm, v16)


# ----------------------------------------------------------------------------
# device program
# ----------------------------------------------------------------------------

def _logits_stage(nc, pools, w, d, as_view, ad_tile):
    """logits + leaky-relu + exp for window w; returns fp16 exp-weight tile.
    Emitted one window ahead of _agg_stage so the ACT latency is hidden by
    the previous window's aggregation work on DVE."""
    spool = pools["s"]
    lp = spool.tile([128, d, 4], F16, tag="lp")
    nc.gpsimd.tensor_add(lp[:], as_view,
                         ad_tile[:, 4 * w:4 * w + 4].unsqueeze(1).broadcast_to([128, d, 4]))
    ll = spool.tile([128, d, 4], F16, tag="ll")
    nc.vector.scalar_tensor_tensor(ll[:], lp[:], 0.2, lp[:],
                                   mybir.AluOpType.mult, mybir.AluOpType.max)
    ew = spool.tile([128, d, 4], F16, tag="ew")
    nc.scalar.activation(ew[:], ll[:], mybir.ActivationFunctionType.Exp)
    return ew


def _agg_half(nc, eng, wpool, spool, d, xg, ew, agg, f0, f1, wtag):
    """Weighted sum over slots for feature range [f0,f1) on engine `eng`."""
    nf = f1 - f0
    first = True
    for j0 in range(0, d, DCAP):
        dc = min(DCAP, d - j0)
        wm = wpool.tile([128, dc, nf], F16, tag=wtag)
        xv = xg[:, j0:j0 + dc, f0:f1].rearrange("p j (c h) -> p j c h", h=HEADS)
        eb = ew[:, j0:j0 + dc, :].unsqueeze(2).broadcast_to([128, dc, nf // HEADS, HEADS])
        eng.tensor_mul(wm[:].rearrange("p j (c h) -> p j c h", h=HEADS), xv, eb)
        # pairwise fp16 tree-sum over slots: stride-1 innermost keeps the DVE
        # 2x packed mode, unlike the transposed tensor_reduce (1x)
        n = dc
        cur = wm
        while n >= 4:
            h = n // 2
            if cur is wm:
                nxt = wpool.tile([128, h, nf], F16, tag=wtag + "t")
                eng.tensor_add(nxt[:], wm[:, 0:h, :], wm[:, h:2 * h, :])
            else:
                nxt = cur
                eng.tensor_add(nxt[:, 0:h, :], cur[:, 0:h, :], cur[:, h:2 * h, :])
            if n % 2:
                eng.tensor_add(nxt[:, 0:1, :], nxt[:, 0:1, :], cur[:, 2 * h:n, :])
            cur, n = nxt, h
        if n == 3:
            eng.tensor_add(cur[:, 0:1, :], cur[:, 0:1, :], cur[:, 2:3, :])
            n = 2
        dst = agg[:, f0:f1]
        if first:
            # write agg slice directly, no intermediate copy
            if n == 1:
                eng.tensor_copy(dst, cur[:, 0:1, :].rearrange("p a c -> p (a c)"))
            else:
                eng.tensor_add(dst, cur[:, 0:1, :].rearrange("p a c -> p (a c)"),
                               cur[:, 1:2, :].rearrange("p a c -> p (a c)"))
            first = False
        else:
            ac = spool.tile([128, nf], F32, tag=wtag + "c")
            if n == 1:
                eng.tensor_copy(ac[:], cur[:, 0:1, :].rearrange("p a c -> p (a c)"))
            else:
                eng.tensor_add(ac[:], cur[:, 0:1, :].rearrange("p a c -> p (a c)"),
                               cur[:, 1:2, :].rearrange("p a c -> p (a c)"))
            eng.tensor_add(dst, dst, ac[:])


def _agg_stage(nc, pools, w, d, xg, ew, den_tile, aggn_cb, fsplit=128):
    spool, wpool = pools["s"], pools["w"]

    nc.vector.tensor_reduce(den_tile[:, 4 * w:4 * w + 4], ew[:].transpose([0, 2, 1]),
                            mybir.AxisListType.X, mybir.AluOpType.add)

    agg = spool.tile([128, 128], F32, tag="agg")
    _agg_half(nc, nc.vector, wpool, spool, d, xg, ew, agg, 0, fsplit, "wm")
    if fsplit < 128:
        _agg_half(nc, nc.gpsimd, wpool, spool, d, xg, ew, agg, fsplit, 128, "wp")

    rec = spool.tile([128, 4], F32, tag="rec")
    nc.vector.reciprocal(rec[:], den_tile[:, 4 * w:4 * w + 4])
    aggn = spool.tile([128, 128], F32, tag="aggn")
    nc.vector.tensor_mul(aggn[:].rearrange("p (c h) -> p c h", h=HEADS),
                         agg[:].rearrange("p (c h) -> p c h", h=HEADS),
                         rec[:].unsqueeze(1).broadcast_to([128, HID, HEADS]))
    aggn_cb(w, aggn)


def _build_program(meta):
    d_w, cs, tot = meta["d_w"], meta["cs"], meta["tot"]

    sim = bool(int(os.environ.get("GAT_SIM", "0")))
    nc = bacc.Bacc("TRN2", num_devices=1 if sim else NCORES,
                   num_swdge_queues=NQ)

    xeT = nc.dram_tensor("xeT", [128, tot * 128], F16, kind="ExternalInput")
    xs = nc.dram_tensor("xs", [128, NPAD], F16, kind="ExternalInput")
    rhs1_h = nc.dram_tensor("rhs1", [128, 132], F16, kind="ExternalInput")
    wdx1_h = nc.dram_tensor("wdx1", [128, 4], F16, kind="ExternalInput")
    rhs2_h = nc.dram_tensor("rhs2", [128, 136], F32, kind="ExternalInput")
    wout_h = nc.dram_tensor("woutd", [128, 8], F32, kind="ExternalInput")
    b1t_h = nc.dram_tensor("b1t", [128, 128], F32, kind="ExternalInput")
    bft_h = nc.dram_tensor("bft", [128, 8], F32, kind="ExternalInput")
    cstm_h = nc.dram_tensor("cstm", [128, 4], F32, kind="ExternalInput")
    ident_h = nc.dram_tensor("ident", [128, 128], F32, kind="ExternalInput")
    i2_h = nc.dram_tensor("i2", [128, 8 * tot], I16, kind="ExternalInput")
    padrow_h = nc.dram_tensor("padrow", [1, ROWW], F16, kind="ExternalInput")

    outy = nc.dram_tensor("outy", [128, WPC * OUTF], F32, kind="ExternalOutput")

    AGIN = nc.dram_tensor("AGIN", [NPAD, ROWW], F16, kind="Internal")
    T2 = nc.dram_tensor("T2", [TB2, ROWW], F16, kind="Internal",
                        addr_space="Local" if sim else "Shared")

    with ExitStack() as ctx:
        tc = ctx.enter_context(tile.TileContext(nc))
        cpool = ctx.enter_context(tc.tile_pool(name="consts", bufs=1))
        pers = ctx.enter_context(tc.tile_pool(name="pers", bufs=1))
        strpool = ctx.enter_context(tc.tile_pool(name="stream", bufs=3))
        gpool = ctx.enter_context(tc.tile_pool(name="gather", bufs=2))
        wpool = ctx.enter_context(tc.tile_pool(name="work", bufs=2))
        spool = ctx.enter_context(tc.tile_pool(name="small", bufs=3))
        pspool = ctx.enter_context(tc.tile_pool(name="ps", bufs=4, space="PSUM"))
        ptpool = ctx.enter_context(tc.tile_pool(name="pt", bufs=2, space="PSUM"))
        pools = {"s": spool, "w": wpool}

        def const(h, shape, dtype=F32, tag=None):
            t = cpool.tile(shape, dtype, tag=tag)
            nc.sync.dma_start(t[:], h[:])
            return t

        rhs1_t = const(rhs1_h, [128, 132], F16, tag="rhs1")
        wdx1_t = const(wdx1_h, [128, 4], F16, tag="wdx1")
        rhs2_t = const(rhs2_h, [128, 136], tag="rhs2")
        wout_t = const(wout_h, [128, 8], tag="wout")
        b1t_t = const(b1t_h, [128, 128], tag="b1t")
        bft_t = const(bft_h, [128, 8], tag="bft")
        cstm_t = const(cstm_h, [128, 4], tag="cstm")
        ident_t = const(ident_h, [128, 128], tag="identc")
        i2_t = const(i2_h, [128, 8 * tot], I16, tag="i2")

        ad1 = pers.tile([128, 4 * WPC], F32)
        ad2 = pers.tile([128, 4 * WPC], F32)
        den1 = pers.tile([128, 4 * WPC], F32)
        den2 = pers.tile([128, 4 * WPC], F32)
        fin = pers.tile([128, OUTF * WPC], F32)

        # ---- a_d1 for owned (sorted) nodes: one bulk DMA + batched matmuls ----
        xs_t = pers.tile([128, NPAD], F16)
        nc.sync.dma_start(xs_t[:], xs[:])
        psA = pspool.tile([128, 4 * WPC], F32, tag="mm")
        for w in range(WPC):
            nc.tensor.matmul(psA[:, 4 * w:4 * w + 4], xs_t[:, w * 128:(w + 1) * 128],
                             wdx1_t[:], start=True, stop=True)
        nc.vector.tensor_copy(ad1[:], psA[:])

        stop = os.environ.get("GAT_STOP", "full")

        # ---- layer 1: stream x[src] cells, matmul h1+a_s1, softmax-agg ----
        def tail1(w, aggn):
            t = spool.tile([128, 128], F32, tag="t1t")
            nc.gpsimd.tensor_add(t[:], aggn[:], b1t_t[:])
            mn = spool.tile([128, 128], F32, tag="t1m")
            nc.gpsimd.tensor_scalar_min(mn[:], t[:], 0.0)
            ex = spool.tile([128, 128], F32, tag="t1e")
            nc.scalar.activation(ex[:], mn[:], mybir.ActivationFunctionType.Exp)
            # y = elu(t) + 1 = max(t,0) + exp(min(t,0)); -1 folded downstream
            y = spool.tile([128, 128], F32, tag="t1x")
            nc.vector.scalar_tensor_tensor(y[:], t[:], 0.0, ex[:],
                                           mybir.AluOpType.max, mybir.AluOpType.add)
            yt_ps = ptpool.tile([128, 128], F32, tag="tr")
            nc.tensor.transpose(yt_ps[:], y[:], ident_t[:])
            yt = spool.tile([128, 128], F32, tag="t1xt")
            nc.scalar.copy(yt[:], yt_ps[:])
            ps = ptpool.tile([128, 136], F32, tag="psb")
            nc.tensor.matmul(ps[:], yt[:], rhs2_t[:], start=True, stop=True)
            rowt = spool.tile([128, 128], F32, tag="rowt")
            nc.scalar.copy(rowt[:].bitcast(F16)[:, 0:128], ps[:, 0:128])
            nc.vector.tensor_copy(rowt[:].bitcast(F16)[:, 128:132], ps[:, 128:132])
            nc.vector.memset(rowt[:, 66:128], 0.0)
            nc.vector.tensor_add(ad2[:, 4 * w:4 * w + 4], ps[:, 132:136], cstm_t[:])
            nc.sync.dma_start(
                AGIN[w * 128:(w + 1) * 128, :].rearrange("(a p) r -> p a r", p=128),
                rowt[:].bitcast(F16).rearrange("p (a r) -> p a r", a=1))

        if stop != "a":
            # stream pieces: piece p covers chunks [p*PIECE, (p+1)*PIECE)
            piece_tiles = {}
            pend1 = None

            def get_piece(p):
                if p in piece_tiles:
                    return piece_tiles[p]
                k = min(PIECE, tot - p * PIECE)
                pt = strpool.tile([128, 128 * PIECE], F16, tag="xep")
                nc.sync.dma_start(pt[:, 0:128 * k],
                                  xeT[:, p * PIECE * 128:(p * PIECE + k) * 128])
                piece_tiles[p] = pt
                return pt

            nbatch = 0
            for w in range(WPC):
                d = int(d_w[w])
                cw = int(cs[w])
                xg = gpool.tile([128, d, L1W], F16, tag="xg1")
                # h1 + a_s1 per slot-column, batched 3 chunks per PSUM tile
                for j0 in range(0, d, 3):
                    bn = min(3, d - j0)
                    ps = pspool.tile([128, 132 * bn], F32, tag="mm")
                    for k in range(bn):
                        g = cw + j0 + k
                        pt = get_piece(g // PIECE)
                        off = (g % PIECE) * 128
                        nc.tensor.matmul(ps[:, 132 * k:132 * (k + 1)],
                                         pt[:, off:off + 128], rhs1_t[:],
                                         start=True, stop=True)
                    nc.scalar.copy(
                        xg[:, j0:j0 + bn, :],
                        ps[:].rearrange("p (b c) -> p b c", b=bn))
                ew = _logits_stage(nc, pools, w, d, xg[:, :, 128:132], ad1)
                if pend1 is not None:
                    _agg_stage(nc, pools, *pend1, den1, tail1)
                pend1 = (w, d, xg, ew)
            if pend1 is not None:
                _agg_stage(nc, pools, *pend1, den1, tail1)

        if stop in ("a", "l1"):
            nc.vector.memset(fin[:], 0.0)
        elif sim:
            nc.sync.dma_start(T2[0:NPAD, :], AGIN[:])
        else:
            nc.gpsimd.collective_compute(
                "AllGather", mybir.AluOpType.bypass,
                replica_groups=[list(range(NCORES))],
                ins=[AGIN[:].opt()], outs=[T2[:].opt()])
        if stop == "full":
            # pad row referenced by all pad cells: zero h2, a_s2 = NEGC
            nc.sync.dma_start(T2[PADROW:PADROW + 1, :], padrow_h[:])

        # ---- layer 2: dma_gather rows from T2, softmax-agg, project ----
        def tail2(w, aggn):
            at_ps = ptpool.tile([128, 128], F32, tag="tr")
            nc.tensor.transpose(at_ps[:], aggn[:], ident_t[:])
            at = spool.tile([128, 128], F32, tag="t2at")
            nc.scalar.copy(at[:], at_ps[:])
            ps8 = ptpool.tile([128, 136], F32, tag="psb")
            nc.tensor.matmul(ps8[:, 0:8], at[:], wout_t[:], start=True, stop=True)
            nc.vector.tensor_add(fin[:, OUTF * w:OUTF * (w + 1)], ps8[:, 0:8], bft_t[:])

        if stop == "ag":
            nc.vector.memset(fin[:], 0.0)
        if stop == "full":
            pend2 = None
            for w in range(WPC):
                d = int(d_w[w])
                cw = int(cs[w])
                xg = gpool.tile([128, d, ROWW], F16, tag="xg2")
                for q, (q0, q1) in enumerate(_qsegs(d)):
                    nq = 128 * (q1 - q0)
                    nc.gpsimd.dma_gather(xg[:, q0:q1, :], T2[BASE:, :],
                                         i2_t[:, 8 * (cw + q0):8 * (cw + q1)],
                                         nq, nq, ROWW, single_packet=False,
                                         queue_num=q)
                ew = _logits_stage(nc, pools, w, d, xg[:, :, 128:132], ad2)
                if pend2 is not None:
                    _agg_stage(nc, pools, *pend2, den2, tail2)
                pend2 = (w, d, xg, ew)
            if pend2 is not None:
                _agg_stage(nc, pools, *pend2, den2, tail2)

        nc.sync.dma_start(outy[:], fin[:])

    nc.compile()
    return nc


# ----------------------------------------------------------------------------
# entry point
# ----------------------------------------------------------------------------

def kernel(x, edge_index, W1, att_src1, att_dst1, b1, W2, att_src2, att_dst2,
           b2, Wout, bout):
    global LAST_RESULT, LAST_NC, LAST_IN_MAPS
    x = np.asarray(x, np.float32)
    edge_index = np.asarray(edge_index)

    ck = hash(edge_index.tobytes())
    if ck not in _CACHE:
        meta, perms = _host_prep(edge_index)
        nc = _build_program(meta)
        _CACHE.clear()
        _CACHE[ck] = (meta, perms, nc)
    meta, perms, nc = _CACHE[ck]
    tot = meta["tot"]

    rhs1, wdx1, rhs2, woutd, b1t, bft, cstm, v16 = _fold_weights(
        np.asarray(W1, np.float32), np.asarray(att_src1, np.float32),
        np.asarray(att_dst1, np.float32), np.asarray(b1, np.float32),
        np.asarray(W2, np.float32), np.asarray(att_src2, np.float32),
        np.asarray(att_dst2, np.float32), np.asarray(b2, np.float32),
        np.asarray(Wout, np.float32), np.asarray(bout, np.float32))

    ident = np.eye(128, dtype=np.float32)
    x16 = x.astype(np.float16)
    padrow = np.zeros((1, ROWW), np.float16)
    padrow[0, 128:132] = NEGC

    in_maps = []
    for c in range(NCORES):
        sc = meta["src_cell"][c]
        xeT = np.tile(v16, (tot * 128, 1))
        real = sc >= 0
        xeT[real] = x16[sc[real]]
        xs = np.zeros((128, NPAD), np.float16)
        xs[:, :NLOC] = x16[c * NLOC + perms[c]].T
        in_maps.append({
            "xeT": np.ascontiguousarray(xeT.T), "xs": xs, "rhs1": rhs1,
            "wdx1": wdx1, "rhs2": rhs2, "woutd": woutd, "b1t": b1t, "bft": bft,
            "cstm": cstm, "ident": ident, "padrow": padrow,
            "i2": np.ascontiguousarray(meta["idxw"][c]),
        })

    trace = bool(int(os.environ.get("GAT_TRACE", "0")))
    res = run_bass_kernel_spmd(nc, in_maps, core_ids=list(range(NCORES)),
                               trace=trace)
    LAST_RESULT = res
    LAST_NC, LAST_IN_MAPS = nc, in_maps

    out = np.empty((N, OUTF), np.float32)
    for c in range(NCORES):
        oy = res.results[c]["outy"].reshape(128, WPC, OUTF)
        oy = oy.transpose(1, 0, 2).reshape(NPAD, OUTF)
        out[c * NLOC + perms[c]] = oy[:NLOC]
    return out


# revision 21
# speedup vs baseline: 1.7450x; 1.7450x over previous
"""GAT (2-layer, 4-head) Trainium2 Bass kernel, 8-core SPMD — v4.

Layer 1: host lays out x[src] in (window, slot, lane) cell order; device
streams it and computes h1 + a_s1 per cell on PE (attention vectors folded
into extra matmul columns). Pad cells hold a vector v with v@wsrc1_h = -160
for every head, so pad logits underflow exp to exact 0 — no mask tensor.
Layer 2: dst-major dma_gather (split across 4 SWDGE queues — the gather is
descriptor-rate-bound per queue) from the AllGather'd T2 row table; pad
cells point at a dedicated pad row storing a_s2 = -160. Rows store
elu(h)+1; the -1 is folded into per-head logit constants and the output
bias (softmax weights sum to 1, so the shift is exact).
Softmax + weighted aggregation run dst-major on DVE; PSUM->SBUF cell
copies alternate between ACT and Pool to balance engine load.
"""

import os
import numpy as np
from contextlib import ExitStack

import concourse.bass as bass
import concourse.tile as tile
from concourse import bacc, mybir
from concourse.bass_utils import run_bass_kernel_spmd

# problem constants (hardcoded per contest contract)
N = 50000
E = 1600000
HEADS = 4
HID = 32
INF = 128
OUTF = 8
NCORES = 8
NLOC = N // NCORES            # 6250 dst per core
WPC = (NLOC + 127) // 128     # 49 windows per core
NPAD = WPC * 128              # 6272
TB2 = NCORES * NPAD           # 50176 rows in layer-2 table
BASE = 17408                  # mid-base for signed int16 gather indices
PADROW = 2 * NPAD + NLOC      # a pad-lane row (zero h, a_s2=-160); idx>=0
ROWW = 256                    # fp16 words per T2 row (512 B)
L1W = 132                     # fp16 words per L1 cell row (h 128 + a_s 4)
DCAP = 32                     # slot-chunk for multiply/reduce working tile
PIECE = 16                    # stream chunks (of 128 cells) per DMA piece
NQ = 4                        # SWDGE queues; L2 window gathers split across
NEGC = -160.0                 # pad logit level (leaky*0.2 -> -32, exp -> 0)

F32 = mybir.dt.float32
F16 = mybir.dt.float16
I16 = mybir.dt.int16

_CACHE = {}
LAST_RESULT = None
LAST_NC = None
LAST_IN_MAPS = None


def _qsegs(d):
    """Split d slots into NQ contiguous per-queue segments (some may be empty)."""
    dq = (d + NQ - 1) // NQ
    segs = []
    for q in range(NQ):
        q0, q1 = q * dq, min(d, (q + 1) * dq)
        if q1 > q0:
            segs.append((q0, q1))
    return segs


# ----------------------------------------------------------------------------
# host-side graph preprocessing
# ----------------------------------------------------------------------------

def _host_prep(edge_index):
    srcs = np.concatenate([edge_index[0], np.arange(N)]).astype(np.int64)
    dsts = np.concatenate([edge_index[1], np.arange(N)]).astype(np.int64)
    ne = srcs.shape[0]

    core = dsts // NLOC
    deg = np.bincount(dsts, minlength=N)

    perms = []
    pos = np.empty(N, np.int64)
    for c in range(NCORES):
        p = np.argsort(-deg[c * NLOC:(c + 1) * NLOC], kind="stable")
        perms.append(p)
        pos[c * NLOC + p] = np.arange(NLOC)

    wpos = pos[dsts]
    w_e = wpos // 128
    lane_e = wpos % 128

    # slot j within each (core, dst) lane, in edge order
    key = core * NLOC + wpos
    order = np.argsort(key, kind="stable")
    ks = key[order]
    change = np.r_[True, ks[1:] != ks[:-1]]
    startpos = np.flatnonzero(change)
    gid = np.cumsum(change) - 1
    j_sorted = np.arange(ne) - startpos[gid]
    j = np.empty(ne, np.int64)
    j[order] = j_sorted

    degs = np.zeros((NCORES, NPAD), np.int64)
    for c in range(NCORES):
        degs[c, :NLOC] = deg[c * NLOC + perms[c]]
    d_w = degs.reshape(NCORES, WPC, 128).max(axis=(0, 2))
    d_w[WPC - 1] = max(int(d_w[WPC - 1]), 1)
    cs = np.r_[0, np.cumsum(d_w)]
    tot = int(cs[-1])
    dmax = int(d_w.max())
    assert dmax <= 80, dmax

    cellpos = (cs[w_e] + j) * 128 + lane_e  # flat cell column per edge

    src_cell = np.full((NCORES, tot * 128), -1, np.int64)
    src_cell[core, cellpos] = srcs

    t2row = (np.arange(N) // NLOC) * NPAD + pos

    idxv = np.full((NCORES, tot * 128), PADROW - BASE, np.int16)
    real = src_cell >= 0
    idxv[real] = (t2row[src_cell[real]] - BASE).astype(np.int16)

    # Trim-safety: Q7 ucode drops trailing NEGATIVE indices of a gather, so
    # the LAST index of every per-queue gather segment must be >= 0 or real
    # cells would be silently dropped. Pads are PADROW-BASE > 0 (safe); for
    # a real negative cell swap slots within lane 127 (slot order within a
    # lane is irrelevant) to put a nonneg-index cell at each segment tail.
    for c in range(NCORES):
        for w in range(WPC):
            cw = int(cs[w])
            d = int(d_w[w])
            tails = {q1 - 1 for _, q1 in _qsegs(d)}
            for tj in sorted(tails):
                lastc = (cw + tj) * 128 + 127
                if idxv[c, lastc] >= 0:
                    continue
                fixed = False
                for jj in range(d):
                    if jj in tails:
                        continue
                    col = (cw + jj) * 128 + 127
                    if idxv[c, col] < 0:
                        continue
                    for arr in (idxv, src_cell):
                        arr[c, col], arr[c, lastc] = arr[c, lastc], arr[c, col]
                    fixed = True
                    break
                assert fixed, f"unfixable trim boundary core {c} window {w}"

    # wrap idx into dma_gather layout [128, 8*tot] (16-partition wrap, 8x rep)
    idxw = np.zeros((NCORES, 128, 8 * tot), np.int16)
    for w in range(WPC):
        d = int(d_w[w])
        cw = int(cs[w])
        blk = idxv[:, cw * 128:(cw + d) * 128]                 # [NC, d*128]
        blk = blk.reshape(NCORES, -1, 16).transpose(0, 2, 1)   # [NC, 16, 8d]
        idxw[:, :, 8 * cw: 8 * (cw + d)] = np.tile(blk, (1, 8, 1))

    return dict(d_w=d_w, cs=cs, tot=tot, dmax=dmax,
                src_cell=src_cell, idxw=idxw), perms


def _fold_weights(W1, att_src1, att_dst1, b1, W2, att_src2, att_dst2, b2, Wout, bout):
    # device feature order is head-interleaved: dev k = c*4 + h <-> ref h*32 + c
    perm = np.array([h * 32 + c for c in range(HID) for h in range(HEADS)])

    def vec(att):  # [HEADS, HID] -> [128, 4] fold in dev space
        z = np.zeros((INF, HEADS), np.float32)
        k = np.arange(INF)
        z[k, k % HEADS] = att[k % HEADS, k // HEADS]
        return z

    W1d = W1[:, perm].astype(np.float64)
    wsx1 = W1d @ vec(att_src1).astype(np.float64)                  # [128,4]
    rhs1 = np.concatenate([W1d, wsx1], axis=1).astype(np.float16)  # [128,132]
    wdx1 = (W1d @ vec(att_dst1).astype(np.float64)).astype(np.float16)

    # pad-cell vector: v @ wsx1_h = NEGC for every head (min-norm solution)
    v = (wsx1 @ np.linalg.solve(wsx1.T @ wsx1, np.full(HEADS, NEGC)))
    v16 = v.astype(np.float16)

    W2d = W2[perm][:, perm].astype(np.float64)
    ws2 = W2d @ vec(att_src2).astype(np.float64)
    wd2 = W2d @ vec(att_dst2).astype(np.float64)
    rhs2 = np.concatenate([W2d, ws2, wd2], axis=1).astype(np.float32)  # [128,136]
    woutd = Wout[perm].astype(np.float64)                               # [128,8]

    cstm = np.zeros((128, 4), np.float32)
    b1t = np.tile(b1[perm].astype(np.float32), (128, 1))               # [128,128]
    bf = b2 @ Wout + bout
    bft = np.tile(bf.astype(np.float32), (128, 1))                     # [128,8]
    return (rhs1, wdx1, rhs2, woutd.astype(np.float32), b1t, bft, cstm, v16)


# ----------------------------------------------------------------------------
# device program
# ----------------------------------------------------------------------------

def _logits_stage(nc, pools, w, d, as_view, ad_tile):
    """logits + leaky-relu + exp for window w; returns fp16 exp-weight tile.
    Emitted one window ahead of _agg_stage so the ACT latency is hidden by
    the previous window's aggregation work on DVE."""
    spool = pools["s"]
    lp = spool.tile([128, d, 4], F16, tag="lp")
    nc.vector.tensor_add(lp[:], as_view,
                         ad_tile[:, 4 * w:4 * w + 4].unsqueeze(1).broadcast_to([128, d, 4]))
    ll = spool.tile([128, d, 4], F16, tag="ll")
    nc.vector.scalar_tensor_tensor(ll[:], lp[:], 0.2, lp[:],
                                   mybir.AluOpType.mult, mybir.AluOpType.max)
    ew = spool.tile([128, d, 4], F16, tag="ew")
    nc.scalar.activation(ew[:], ll[:], mybir.ActivationFunctionType.Exp)
    return ew


def _agg_half(nc, eng, wpool, spool, d, xg, ew, agg, f0, f1, wtag):
    """Weighted sum over slots for feature range [f0,f1) on engine `eng`."""
    nf = f1 - f0
    first = True
    for j0 in range(0, d, DCAP):
        dc = min(DCAP, d - j0)
        wm = wpool.tile([128, dc, nf], F16, tag=wtag)
        xv = xg[:, j0:j0 + dc, f0:f1].rearrange("p j (c h) -> p j c h", h=HEADS)
        eb = ew[:, j0:j0 + dc, :].unsqueeze(2).broadcast_to([128, dc, nf // HEADS, HEADS])
        eng.tensor_mul(wm[:].rearrange("p j (c h) -> p j c h", h=HEADS), xv, eb)
        # pairwise fp16 tree-sum over slots: stride-1 innermost keeps the DVE
        # 2x packed mode, unlike the transposed tensor_reduce (1x)
        n = dc
        cur = wm
        while n >= 4:
            h = n // 2
            if cur is wm:
                nxt = wpool.tile([128, h, nf], F16, tag=wtag + "t")
                eng.tensor_add(nxt[:], wm[:, 0:h, :], wm[:, h:2 * h, :])
            else:
                nxt = cur
                eng.tensor_add(nxt[:, 0:h, :], cur[:, 0:h, :], cur[:, h:2 * h, :])
            if n % 2:
                eng.tensor_add(nxt[:, 0:1, :], nxt[:, 0:1, :], cur[:, 2 * h:n, :])
            cur, n = nxt, h
        if n == 3:
            eng.tensor_add(cur[:, 0:1, :], cur[:, 0:1, :], cur[:, 2:3, :])
            n = 2
        dst = agg[:, f0:f1]
        if first:
            # write agg slice directly, no intermediate copy
            if n == 1:
                eng.tensor_copy(dst, cur[:, 0:1, :].rearrange("p a c -> p (a c)"))
            else:
                eng.tensor_add(dst, cur[:, 0:1, :].rearrange("p a c -> p (a c)"),
                               cur[:, 1:2, :].rearrange("p a c -> p (a c)"))
            first = False
        else:
            ac = spool.tile([128, nf], F32, tag=wtag + "c")
            if n == 1:
                eng.tensor_copy(ac[:], cur[:, 0:1, :].rearrange("p a c -> p (a c)"))
            else:
                eng.tensor_add(ac[:], cur[:, 0:1, :].rearrange("p a c -> p (a c)"),
                               cur[:, 1:2, :].rearrange("p a c -> p (a c)"))
            eng.tensor_add(dst, dst, ac[:])


def _agg_stage(nc, pools, w, d, xg, ew, den_tile, aggn_cb, fsplit=128):
    spool, wpool = pools["s"], pools["w"]

    nc.vector.tensor_reduce(den_tile[:, 4 * w:4 * w + 4], ew[:].transpose([0, 2, 1]),
                            mybir.AxisListType.X, mybir.AluOpType.add)

    agg = spool.tile([128, 128], F32, tag="agg")
    _agg_half(nc, nc.vector, wpool, spool, d, xg, ew, agg, 0, fsplit, "wm")
    if fsplit < 128:
        _agg_half(nc, nc.gpsimd, wpool, spool, d, xg, ew, agg, fsplit, 128, "wp")

    rec = spool.tile([128, 4], F32, tag="rec")
    nc.vector.reciprocal(rec[:], den_tile[:, 4 * w:4 * w + 4])
    aggn = spool.tile([128, 128], F32, tag="aggn")
    nc.vector.tensor_mul(aggn[:].rearrange("p (c h) -> p c h", h=HEADS),
                         agg[:].rearrange("p (c h) -> p c h", h=HEADS),
                         rec[:].unsqueeze(1).broadcast_to([128, HID, HEADS]))
    aggn_cb(w, aggn)


def _build_program(meta):
    d_w, cs, tot = meta["d_w"], meta["cs"], meta["tot"]

    sim = bool(int(os.environ.get("GAT_SIM", "0")))
    nc = bacc.Bacc("TRN2", num_devices=1 if sim else NCORES,
                   num_swdge_queues=NQ)

    xeT = nc.dram_tensor("xeT", [128, tot * 128], F16, kind="ExternalInput")
    xs = nc.dram_tensor("xs", [128, NPAD], F16, kind="ExternalInput")
    rhs1_h = nc.dram_tensor("rhs1", [128, 132], F16, kind="ExternalInput")
    wdx1_h = nc.dram_tensor("wdx1", [128, 4], F16, kind="ExternalInput")
    rhs2_h = nc.dram_tensor("rhs2", [128, 136], F32, kind="ExternalInput")
    wout_h = nc.dram_tensor("woutd", [128, 8], F32, kind="ExternalInput")
    b1t_h = nc.dram_tensor("b1t", [128, 128], F32, kind="ExternalInput")
    bft_h = nc.dram_tensor("bft", [128, 8], F32, kind="ExternalInput")
    cstm_h = nc.dram_tensor("cstm", [128, 4], F32, kind="ExternalInput")
    ident_h = nc.dram_tensor("ident", [128, 128], F32, kind="ExternalInput")
    i2_h = nc.dram_tensor("i2", [128, 8 * tot], I16, kind="ExternalInput")
    padrow_h = nc.dram_tensor("padrow", [1, ROWW], F16, kind="ExternalInput")

    outy = nc.dram_tensor("outy", [128, WPC * OUTF], F32, kind="ExternalOutput")

    AGIN = nc.dram_tensor("AGIN", [NPAD, ROWW], F16, kind="Internal")
    T2 = nc.dram_tensor("T2", [TB2, ROWW], F16, kind="Internal",
                        addr_space="Local" if sim else "Shared")

    with ExitStack() as ctx:
        tc = ctx.enter_context(tile.TileContext(nc))
        cpool = ctx.enter_context(tc.tile_pool(name="consts", bufs=1))
        pers = ctx.enter_context(tc.tile_pool(name="pers", bufs=1))
        strpool = ctx.enter_context(tc.tile_pool(name="stream", bufs=3))
        gpool = ctx.enter_context(tc.tile_pool(name="gather", bufs=2))
        wpool = ctx.enter_context(tc.tile_pool(name="work", bufs=2))
        spool = ctx.enter_context(tc.tile_pool(name="small", bufs=3))
        pspool = ctx.enter_context(tc.tile_pool(name="ps", bufs=4, space="PSUM"))
        ptpool = ctx.enter_context(tc.tile_pool(name="pt", bufs=2, space="PSUM"))
        pools = {"s": spool, "w": wpool}

        def const(h, shape, dtype=F32, tag=None):
            t = cpool.tile(shape, dtype, tag=tag)
            nc.sync.dma_start(t[:], h[:])
            return t

        rhs1_t = const(rhs1_h, [128, 132], F16, tag="rhs1")
        wdx1_t = const(wdx1_h, [128, 4], F16, tag="wdx1")
        rhs2_t = const(rhs2_h, [128, 136], tag="rhs2")
        wout_t = const(wout_h, [128, 8], tag="wout")
        b1t_t = const(b1t_h, [128, 128], tag="b1t")
        bft_t = const(bft_h, [128, 8], tag="bft")
        cstm_t = const(cstm_h, [128, 4], tag="cstm")
        ident_t = const(ident_h, [128, 128], tag="identc")
        i2_t = const(i2_h, [128, 8 * tot], I16, tag="i2")

        ad1 = pers.tile([128, 4 * WPC], F32)
        ad2 = pers.tile([128, 4 * WPC], F32)
        den1 = pers.tile([128, 4 * WPC], F32)
        den2 = pers.tile([128, 4 * WPC], F32)
        fin = pers.tile([128, OUTF * WPC], F32)

        # ---- a_d1 for owned (sorted) nodes: one bulk DMA + batched matmuls ----
        xs_t = pers.tile([128, NPAD], F16)
        nc.sync.dma_start(xs_t[:], xs[:])
        psA = pspool.tile([128, 4 * WPC], F32, tag="mm")
        for w in range(WPC):
            nc.tensor.matmul(psA[:, 4 * w:4 * w + 4], xs_t[:, w * 128:(w + 1) * 128],
                             wdx1_t[:], start=True, stop=True)
        nc.vector.tensor_copy(ad1[:], psA[:])

        stop = os.environ.get("GAT_STOP", "full")

        # ---- layer 1: stream x[src] cells, matmul h1+a_s1, softmax-agg ----
        def tail1(w, aggn):
            t = spool.tile([128, 128], F32, tag="t1t")
            nc.vector.tensor_add(t[:], aggn[:], b1t_t[:])
            mn = spool.tile([128, 128], F32, tag="t1m")
            nc.vector.tensor_scalar_min(mn[:], t[:], 0.0)
            ex = spool.tile([128, 128], F32, tag="t1e")
            nc.scalar.activation(ex[:], mn[:], mybir.ActivationFunctionType.Exp)
            y = spool.tile([128, 128], F32, tag="t1x")
            nc.vector.scalar_tensor_tensor(y[:], t[:], 0.0, ex[:],
                                           mybir.AluOpType.max, mybir.AluOpType.add)
            nc.vector.tensor_scalar_sub(y[:], y[:], 1.0)
            yt_ps = ptpool.tile([128, 128], F32, tag="tr")
            nc.tensor.transpose(yt_ps[:], y[:], ident_t[:])
            yt = spool.tile([128, 128], F32, tag="t1xt")
            nc.scalar.copy(yt[:], yt_ps[:])
            ps = ptpool.tile([128, 136], F32, tag="psb")
            nc.tensor.matmul(ps[:], yt[:], rhs2_t[:], start=True, stop=True)
            rowt = spool.tile([128, 128], F32, tag="rowt")
            nc.scalar.copy(rowt[:].bitcast(F16)[:, 0:128], ps[:, 0:128])
            nc.vector.tensor_copy(rowt[:].bitcast(F16)[:, 128:132], ps[:, 128:132])
            nc.vector.memset(rowt[:, 66:128], 0.0)
            nc.vector.tensor_copy(ad2[:, 4 * w:4 * w + 4], ps[:, 132:136])
            nc.sync.dma_start(
                AGIN[w * 128:(w + 1) * 128, :].rearrange("(a p) r -> p a r", p=128),
                rowt[:].bitcast(F16).rearrange("p (a r) -> p a r", a=1))

        if stop != "a":
            # stream pieces: piece p covers chunks [p*PIECE, (p+1)*PIECE)
            piece_tiles = {}
            pend1 = None

            def get_piece(p):
                if p in piece_tiles:
                    return piece_tiles[p]
                k = min(PIECE, tot - p * PIECE)
                pt = strpool.tile([128, 128 * PIECE], F16, tag="xep")
                nc.sync.dma_start(pt[:, 0:128 * k],
                                  xeT[:, p * PIECE * 128:(p * PIECE + k) * 128])
                piece_tiles[p] = pt
                return pt

            nbatch = 0
            for w in range(WPC):
                d = int(d_w[w])
                cw = int(cs[w])
                xg = gpool.tile([128, d, L1W], F16, tag="xg1")
                # h1 + a_s1 per slot-column, batched 3 chunks per PSUM tile
                for j0 in range(0, d, 3):
                    bn = min(3, d - j0)
                    ps = pspool.tile([128, 132 * bn], F32, tag="mm")
                    for k in range(bn):
                        g = cw + j0 + k
                        pt = get_piece(g // PIECE)
                        off = (g % PIECE) * 128
                        nc.tensor.matmul(ps[:, 132 * k:132 * (k + 1)],
                                         pt[:, off:off + 128], rhs1_t[:],
                                         start=True, stop=True)
                    nc.scalar.copy(
                        xg[:, j0:j0 + bn, :],
                        ps[:].rearrange("p (b c) -> p b c", b=bn))
                ew = _logits_stage(nc, pools, w, d, xg[:, :, 128:132], ad1)
                if pend1 is not None:
                    _agg_stage(nc, pools, *pend1, den1, tail1)
                pend1 = (w, d, xg, ew)
            if pend1 is not None:
                _agg_stage(nc, pools, *pend1, den1, tail1)

        if stop in ("a", "l1"):
            nc.vector.memset(fin[:], 0.0)
        elif sim:
            nc.sync.dma_start(T2[0:NPAD, :], AGIN[:])
        else:
            nc.gpsimd.collective_compute(
                "AllGather", mybir.AluOpType.bypass,
                replica_groups=[list(range(NCORES))],
                ins=[AGIN[:].opt()], outs=[T2[:].opt()])
        if stop == "full":
            # pad row referenced by all pad cells: zero h2, a_s2 = NEGC
            nc.sync.dma_start(T2[PADROW:PADROW + 1, :], padrow_h[:])

        # ---- layer 2: dma_gather rows from T2, softmax-agg, project ----
        def tail2(w, aggn):
            at_ps = ptpool.tile([128, 128], F32, tag="tr")
            nc.tensor.transpose(at_ps[:], aggn[:], ident_t[:])
            at = spool.tile([128, 128], F32, tag="t2at")
            nc.scalar.copy(at[:], at_ps[:])
            ps8 = ptpool.tile([128, 136], F32, tag="psb")
            nc.tensor.matmul(ps8[:, 0:8], at[:], wout_t[:], start=True, stop=True)
            nc.vector.tensor_add(fin[:, OUTF * w:OUTF * (w + 1)], ps8[:, 0:8], bft_t[:])

        if stop == "ag":
            nc.vector.memset(fin[:], 0.0)
        if stop == "full":
            pend2 = None
            for w in range(WPC):
                d = int(d_w[w])
                cw = int(cs[w])
                xg = gpool.tile([128, d, ROWW], F16, tag="xg2")
                for q, (q0, q1) in enumerate(_qsegs(d)):
                    nq = 128 * (q1 - q0)
                    nc.gpsimd.dma_gather(xg[:, q0:q1, :], T2[BASE:, :],
                                         i2_t[:, 8 * (cw + q0):8 * (cw + q1)],
                                         nq, nq, ROWW, single_packet=False,
                                         queue_num=q)
                ew = _logits_stage(nc, pools, w, d, xg[:, :, 128:132], ad2)
                if pend2 is not None:
                    _agg_stage(nc, pools, *pend2, den2, tail2)
                pend2 = (w, d, xg, ew)
            if pend2 is not None:
                _agg_stage(nc, pools, *pend2, den2, tail2)

        nc.sync.dma_start(outy[:], fin[:])

    nc.compile()
    return nc


# ----------------------------------------------------------------------------
# entry point
# ----------------------------------------------------------------------------

def kernel(x, edge_index, W1, att_src1, att_dst1, b1, W2, att_src2, att_dst2,
           b2, Wout, bout):
    global LAST_RESULT, LAST_NC, LAST_IN_MAPS
    x = np.asarray(x, np.float32)
    edge_index = np.asarray(edge_index)

    ck = hash(edge_index.tobytes())
    if ck not in _CACHE:
        meta, perms = _host_prep(edge_index)
        nc = _build_program(meta)
        _CACHE.clear()
        _CACHE[ck] = (meta, perms, nc)
    meta, perms, nc = _CACHE[ck]
    tot = meta["tot"]

    rhs1, wdx1, rhs2, woutd, b1t, bft, cstm, v16 = _fold_weights(
        np.asarray(W1, np.float32), np.asarray(att_src1, np.float32),
        np.asarray(att_dst1, np.float32), np.asarray(b1, np.float32),
        np.asarray(W2, np.float32), np.asarray(att_src2, np.float32),
        np.asarray(att_dst2, np.float32), np.asarray(b2, np.float32),
        np.asarray(Wout, np.float32), np.asarray(bout, np.float32))

    ident = np.eye(128, dtype=np.float32)
    x16 = x.astype(np.float16)
    padrow = np.zeros((1, ROWW), np.float16)
    padrow[0, 128:132] = NEGC

    in_maps = []
    for c in range(NCORES):
        sc = meta["src_cell"][c]
        xeT = np.tile(v16, (tot * 128, 1))
        real = sc >= 0
        xeT[real] = x16[sc[real]]
        xs = np.zeros((128, NPAD), np.float16)
        xs[:, :NLOC] = x16[c * NLOC + perms[c]].T
        in_maps.append({
            "xeT": np.ascontiguousarray(xeT.T), "xs": xs, "rhs1": rhs1,
            "wdx1": wdx1, "rhs2": rhs2, "woutd": woutd, "b1t": b1t, "bft": bft,
            "cstm": cstm, "ident": ident, "padrow": padrow,
            "i2": np.ascontiguousarray(meta["idxw"][c]),
        })

    trace = bool(int(os.environ.get("GAT_TRACE", "0")))
    res = run_bass_kernel_spmd(nc, in_maps, core_ids=list(range(NCORES)),
                               trace=trace)
    LAST_RESULT = res
    LAST_NC, LAST_IN_MAPS = nc, in_maps

    out = np.empty((N, OUTF), np.float32)
    for c in range(NCORES):
        oy = res.results[c]["outy"].reshape(128, WPC, OUTF)
        oy = oy.transpose(1, 0, 2).reshape(NPAD, OUTF)
        out[c * NLOC + perms[c]] = oy[:NLOC]
    return out


# revision 25
# speedup vs baseline: 1.9251x; 1.1032x over previous
"""GAT (2-layer, 4-head) Trainium2 Bass kernel, 8-core SPMD — v4.

Layer 1: host lays out x[src] in (window, slot, lane) cell order; device
streams it and computes h1 + a_s1 per cell on PE (attention vectors folded
into extra matmul columns). Pad cells hold a vector v with v@wsrc1_h = -160
for every head, so pad logits underflow exp to exact 0 — no mask tensor.
Layer 2: dst-major dma_gather (split across 4 SWDGE queues — the gather is
descriptor-rate-bound per queue) from the AllGather'd T2 row table; pad
cells point at a dedicated pad row storing a_s2 = -160. Rows store
elu(h)+1; the -1 is folded into per-head logit constants and the output
bias (softmax weights sum to 1, so the shift is exact).
Softmax + weighted aggregation run dst-major on DVE; PSUM->SBUF cell
copies alternate between ACT and Pool to balance engine load.
"""

import os
import numpy as np
from contextlib import ExitStack

import concourse.bass as bass
import concourse.tile as tile
from concourse import bacc, mybir
from concourse.bass_utils import run_bass_kernel_spmd

# problem constants (hardcoded per contest contract)
N = 50000
E = 1600000
HEADS = 4
HID = 32
INF = 128
OUTF = 8
NCORES = 8
NLOC = N // NCORES            # 6250 dst per core
WPC = (NLOC + 127) // 128     # 49 windows per core
NPAD = WPC * 128              # 6272
TB2 = NCORES * NPAD           # 50176 rows in layer-2 table
BASE = 17408                  # mid-base for signed int16 gather indices
PADROW = 2 * NPAD + NLOC      # a pad-lane row (zero h, a_s2=-160); idx>=0
ROWW = 256                    # fp16 words per T2 row (512 B)
L1W = 132                     # fp16 words per L1 cell row (h 128 + a_s 4)
DCAP = 32                     # slot-chunk for multiply/reduce working tile
PIECE = 16                    # stream chunks (of 128 cells) per DMA piece
NQ = 4                        # SWDGE queues; L2 window gathers split across
NEGC = -160.0                 # pad logit level (leaky*0.2 -> -32, exp -> 0)

F32 = mybir.dt.float32
F16 = mybir.dt.float16
I16 = mybir.dt.int16

_CACHE = {}
LAST_RESULT = None
LAST_NC = None
LAST_IN_MAPS = None


def _qsegs(d):
    """Split d slots into NQ contiguous per-queue segments (some may be empty)."""
    dq = (d + NQ - 1) // NQ
    segs = []
    for q in range(NQ):
        q0, q1 = q * dq, min(d, (q + 1) * dq)
        if q1 > q0:
            segs.append((q0, q1))
    return segs


# ----------------------------------------------------------------------------
# host-side graph preprocessing
# ----------------------------------------------------------------------------

def _host_prep(edge_index):
    srcs = np.concatenate([edge_index[0], np.arange(N)]).astype(np.int64)
    dsts = np.concatenate([edge_index[1], np.arange(N)]).astype(np.int64)
    ne = srcs.shape[0]

    core = dsts // NLOC
    deg = np.bincount(dsts, minlength=N)

    perms = []
    pos = np.empty(N, np.int64)
    for c in range(NCORES):
        p = np.argsort(-deg[c * NLOC:(c + 1) * NLOC], kind="stable")
        perms.append(p)
        pos[c * NLOC + p] = np.arange(NLOC)

    wpos = pos[dsts]
    w_e = wpos // 128
    lane_e = wpos % 128

    # slot j within each (core, dst) lane, in edge order
    key = core * NLOC + wpos
    order = np.argsort(key, kind="stable")
    ks = key[order]
    change = np.r_[True, ks[1:] != ks[:-1]]
    startpos = np.flatnonzero(change)
    gid = np.cumsum(change) - 1
    j_sorted = np.arange(ne) - startpos[gid]
    j = np.empty(ne, np.int64)
    j[order] = j_sorted

    degs = np.zeros((NCORES, NPAD), np.int64)
    for c in range(NCORES):
        degs[c, :NLOC] = deg[c * NLOC + perms[c]]
    d_w = degs.reshape(NCORES, WPC, 128).max(axis=(0, 2))
    d_w[WPC - 1] = max(int(d_w[WPC - 1]), 1)
    cs = np.r_[0, np.cumsum(d_w)]
    tot = int(cs[-1])
    dmax = int(d_w.max())
    assert dmax <= 80, dmax

    cellpos = (cs[w_e] + j) * 128 + lane_e  # flat cell column per edge

    src_cell = np.full((NCORES, tot * 128), -1, np.int64)
    src_cell[core, cellpos] = srcs

    t2row = (np.arange(N) // NLOC) * NPAD + pos

    idxv = np.full((NCORES, tot * 128), PADROW - BASE, np.int16)
    real = src_cell >= 0
    idxv[real] = (t2row[src_cell[real]] - BASE).astype(np.int16)

    # Trim-safety: Q7 ucode drops trailing NEGATIVE indices of a gather, so
    # the LAST index of every per-queue gather segment must be >= 0 or real
    # cells would be silently dropped. Pads are PADROW-BASE > 0 (safe); for
    # a real negative cell swap slots within lane 127 (slot order within a
    # lane is irrelevant) to put a nonneg-index cell at each segment tail.
    for c in range(NCORES):
        for w in range(WPC):
            cw = int(cs[w])
            d = int(d_w[w])
            tails = {q1 - 1 for _, q1 in _qsegs(d)}
            for tj in sorted(tails):
                lastc = (cw + tj) * 128 + 127
                if idxv[c, lastc] >= 0:
                    continue
                fixed = False
                for jj in range(d):
                    if jj in tails:
                        continue
                    col = (cw + jj) * 128 + 127
                    if idxv[c, col] < 0:
                        continue
                    for arr in (idxv, src_cell):
                        arr[c, col], arr[c, lastc] = arr[c, lastc], arr[c, col]
                    fixed = True
                    break
                assert fixed, f"unfixable trim boundary core {c} window {w}"

    # wrap idx into dma_gather layout [128, 8*tot] (16-partition wrap, 8x rep)
    idxw = np.zeros((NCORES, 128, 8 * tot), np.int16)
    for w in range(WPC):
        d = int(d_w[w])
        cw = int(cs[w])
        blk = idxv[:, cw * 128:(cw + d) * 128]                 # [NC, d*128]
        blk = blk.reshape(NCORES, -1, 16).transpose(0, 2, 1)   # [NC, 16, 8d]
        idxw[:, :, 8 * cw: 8 * (cw + d)] = np.tile(blk, (1, 8, 1))

    return dict(d_w=d_w, cs=cs, tot=tot, dmax=dmax,
                src_cell=src_cell, idxw=idxw), perms


def _fold_weights(W1, att_src1, att_dst1, b1, W2, att_src2, att_dst2, b2, Wout, bout):
    # device feature order is head-interleaved: dev k = c*4 + h <-> ref h*32 + c
    perm = np.array([h * 32 + c for c in range(HID) for h in range(HEADS)])

    def vec(att):  # [HEADS, HID] -> [128, 4] fold in dev space
        z = np.zeros((INF, HEADS), np.float32)
        k = np.arange(INF)
        z[k, k % HEADS] = att[k % HEADS, k // HEADS]
        return z

    W1d = W1[:, perm].astype(np.float64)
    wsx1 = W1d @ vec(att_src1).astype(np.float64)                  # [128,4]
    rhs1 = np.concatenate([W1d, wsx1], axis=1).astype(np.float16)  # [128,132]
    wdx1 = (W1d @ vec(att_dst1).astype(np.float64)).astype(np.float16)

    # pad-cell vector: v @ wsx1_h = NEGC for every head (min-norm solution)
    v = (wsx1 @ np.linalg.solve(wsx1.T @ wsx1, np.full(HEADS, NEGC)))
    v16 = v.astype(np.float16)

    W2d = W2[perm][:, perm].astype(np.float64)
    ws2 = W2d @ vec(att_src2).astype(np.float64)
    wd2 = W2d @ vec(att_dst2).astype(np.float64)
    rhs2 = np.concatenate([W2d, ws2, wd2], axis=1).astype(np.float32)  # [128,136]
    woutd = Wout[perm].astype(np.float64)                               # [128,8]

    cstm = np.zeros((128, 4), np.float32)
    b1t = np.tile(b1[perm].astype(np.float32), (128, 1))               # [128,128]
    bf = b2 @ Wout + bout
    bft = np.tile(bf.astype(np.float32), (128, 1))                     # [128,8]
    return (rhs1, wdx1, rhs2, woutd.astype(np.float32), b1t, bft, cstm, v16)


# ----------------------------------------------------------------------------
# device program
# ----------------------------------------------------------------------------

def _logits_stage(nc, pools, w, d, as_view, ad_tile, leaky_act=False):
    """logits + leaky-relu + exp for window w; returns fp16 exp-weight tile.
    Emitted one window ahead of _agg_stage so the ACT latency is hidden by
    the previous window's aggregation work on DVE. leaky_act=True runs the
    leaky-relu on ACT as Prelu (same act-table set as Exp), for phases where
    DVE is the bottleneck and ACT is idle."""
    spool = pools["s"]
    lp = spool.tile([128, d, 4], F16, tag="lp")
    nc.vector.tensor_add(lp[:], as_view,
                         ad_tile[:, 4 * w:4 * w + 4].unsqueeze(1).broadcast_to([128, d, 4]))
    ll = spool.tile([128, d, 4], F16, tag="ll")
    if leaky_act:
        nc.scalar.activation(ll[:], lp[:], mybir.ActivationFunctionType.Prelu,
                             alpha=0.2)
    else:
        nc.vector.scalar_tensor_tensor(ll[:], lp[:], 0.2, lp[:],
                                       mybir.AluOpType.mult, mybir.AluOpType.max)
    ew = spool.tile([128, d, 4], F16, tag="ew")
    nc.scalar.activation(ew[:], ll[:], mybir.ActivationFunctionType.Exp)
    return ew


def _agg_half(nc, eng, wpool, spool, d, xg, ew, agg, f0, f1, wtag):
    """Weighted sum over slots for feature range [f0,f1) on engine `eng`."""
    nf = f1 - f0
    first = True
    for j0 in range(0, d, DCAP):
        dc = min(DCAP, d - j0)
        wm = wpool.tile([128, dc, nf], F16, tag=wtag)
        xv = xg[:, j0:j0 + dc, f0:f1].rearrange("p j (c h) -> p j c h", h=HEADS)
        eb = ew[:, j0:j0 + dc, :].unsqueeze(2).broadcast_to([128, dc, nf // HEADS, HEADS])
        eng.tensor_mul(wm[:].rearrange("p j (c h) -> p j c h", h=HEADS), xv, eb)
        # pairwise fp16 tree-sum over slots: stride-1 innermost keeps the DVE
        # 2x packed mode, unlike the transposed tensor_reduce (1x)
        n = dc
        cur = wm
        while n >= 4:
            h = n // 2
            if cur is wm:
                nxt = wpool.tile([128, h, nf], F16, tag=wtag + "t")
                eng.tensor_add(nxt[:], wm[:, 0:h, :], wm[:, h:2 * h, :])
            else:
                nxt = cur
                eng.tensor_add(nxt[:, 0:h, :], cur[:, 0:h, :], cur[:, h:2 * h, :])
            if n % 2:
                eng.tensor_add(nxt[:, 0:1, :], nxt[:, 0:1, :], cur[:, 2 * h:n, :])
            cur, n = nxt, h
        if n == 3:
            eng.tensor_add(cur[:, 0:1, :], cur[:, 0:1, :], cur[:, 2:3, :])
            n = 2
        dst = agg[:, f0:f1]
        if first:
            # write agg slice directly, no intermediate copy
            if n == 1:
                eng.tensor_copy(dst, cur[:, 0:1, :].rearrange("p a c -> p (a c)"))
            else:
                eng.tensor_add(dst, cur[:, 0:1, :].rearrange("p a c -> p (a c)"),
                               cur[:, 1:2, :].rearrange("p a c -> p (a c)"))
            first = False
        else:
            ac = spool.tile([128, nf], F32, tag=wtag + "c")
            if n == 1:
                eng.tensor_copy(ac[:], cur[:, 0:1, :].rearrange("p a c -> p (a c)"))
            else:
                eng.tensor_add(ac[:], cur[:, 0:1, :].rearrange("p a c -> p (a c)"),
                               cur[:, 1:2, :].rearrange("p a c -> p (a c)"))
            eng.tensor_add(dst, dst, ac[:])


def _agg_stage(nc, pools, w, d, xg, ew, den_tile, aggn_cb, fsplit=128):
    spool, wpool = pools["s"], pools["w"]

    nc.vector.tensor_reduce(den_tile[:, 4 * w:4 * w + 4], ew[:].transpose([0, 2, 1]),
                            mybir.AxisListType.X, mybir.AluOpType.add)

    agg = spool.tile([128, 128], F32, tag="agg")
    _agg_half(nc, nc.vector, wpool, spool, d, xg, ew, agg, 0, fsplit, "wm")
    if fsplit < 128:
        _agg_half(nc, nc.gpsimd, wpool, spool, d, xg, ew, agg, fsplit, 128, "wp")

    rec = spool.tile([128, 4], F32, tag="rec")
    nc.vector.reciprocal(rec[:], den_tile[:, 4 * w:4 * w + 4])
    aggn = spool.tile([128, 128], F32, tag="aggn")
    nc.vector.tensor_mul(aggn[:].rearrange("p (c h) -> p c h", h=HEADS),
                         agg[:].rearrange("p (c h) -> p c h", h=HEADS),
                         rec[:].unsqueeze(1).broadcast_to([128, HID, HEADS]))
    aggn_cb(w, aggn)


def _build_program(meta):
    d_w, cs, tot = meta["d_w"], meta["cs"], meta["tot"]

    sim = bool(int(os.environ.get("GAT_SIM", "0")))
    nc = bacc.Bacc("TRN2", num_devices=1 if sim else NCORES,
                   num_swdge_queues=NQ)

    xeT = nc.dram_tensor("xeT", [128, tot * 128], F16, kind="ExternalInput")
    xs = nc.dram_tensor("xs", [128, NPAD], F16, kind="ExternalInput")
    rhs1_h = nc.dram_tensor("rhs1", [128, 132], F16, kind="ExternalInput")
    wdx1_h = nc.dram_tensor("wdx1", [128, 4], F16, kind="ExternalInput")
    rhs2_h = nc.dram_tensor("rhs2", [128, 136], F32, kind="ExternalInput")
    wout_h = nc.dram_tensor("woutd", [128, 8], F32, kind="ExternalInput")
    b1t_h = nc.dram_tensor("b1t", [128, 128], F32, kind="ExternalInput")
    bft_h = nc.dram_tensor("bft", [128, 8], F32, kind="ExternalInput")
    cstm_h = nc.dram_tensor("cstm", [128, 4], F32, kind="ExternalInput")
    ident_h = nc.dram_tensor("ident", [128, 128], F32, kind="ExternalInput")
    i2_h = nc.dram_tensor("i2", [128, 8 * tot], I16, kind="ExternalInput")
    padrow_h = nc.dram_tensor("padrow", [1, ROWW], F16, kind="ExternalInput")

    outy = nc.dram_tensor("outy", [128, WPC * OUTF], F32, kind="ExternalOutput")

    AGIN = nc.dram_tensor("AGIN", [NPAD, ROWW], F16, kind="Internal")
    T2 = nc.dram_tensor("T2", [TB2, ROWW], F16, kind="Internal",
                        addr_space="Local" if sim else "Shared")

    with ExitStack() as ctx:
        tc = ctx.enter_context(tile.TileContext(nc))
        cpool = ctx.enter_context(tc.tile_pool(name="consts", bufs=1))
        pers = ctx.enter_context(tc.tile_pool(name="pers", bufs=1))
        strpool = ctx.enter_context(tc.tile_pool(name="stream", bufs=3))
        gpool = ctx.enter_context(tc.tile_pool(name="gather", bufs=2))
        wpool = ctx.enter_context(tc.tile_pool(name="work", bufs=2))
        spool = ctx.enter_context(tc.tile_pool(name="small", bufs=3))
        pspool = ctx.enter_context(tc.tile_pool(name="ps", bufs=2, space="PSUM"))
        ptpool = ctx.enter_context(tc.tile_pool(name="pt", bufs=2, space="PSUM"))
        pools = {"s": spool, "w": wpool}

        def const(h, shape, dtype=F32, tag=None):
            t = cpool.tile(shape, dtype, tag=tag)
            nc.sync.dma_start(t[:], h[:])
            return t

        rhs1_t = const(rhs1_h, [128, 132], F16, tag="rhs1")
        wdx1_t = const(wdx1_h, [128, 4], F16, tag="wdx1")
        rhs2_t = const(rhs2_h, [128, 136], tag="rhs2")
        wout_t = const(wout_h, [128, 8], tag="wout")
        b1t_t = const(b1t_h, [128, 128], tag="b1t")
        bft_t = const(bft_h, [128, 8], tag="bft")
        cstm_t = const(cstm_h, [128, 4], tag="cstm")
        ident_t = const(ident_h, [128, 128], tag="identc")
        i2_t = const(i2_h, [128, 8 * tot], I16, tag="i2")

        ad1 = pers.tile([128, 4 * WPC], F32)
        ad2 = pers.tile([128, 4 * WPC], F32)
        den1 = pers.tile([128, 4 * WPC], F32)
        den2 = pers.tile([128, 4 * WPC], F32)
        fin = pers.tile([128, OUTF * WPC], F32)

        # ---- a_d1 for owned (sorted) nodes: one bulk DMA + batched matmuls ----
        xs_t = pers.tile([128, NPAD], F16)
        nc.sync.dma_start(xs_t[:], xs[:])
        psA = pspool.tile([128, 4 * WPC], F32, tag="mm")
        for w in range(WPC):
            nc.tensor.matmul(psA[:, 4 * w:4 * w + 4], xs_t[:, w * 128:(w + 1) * 128],
                             wdx1_t[:], start=True, stop=True)
        nc.vector.tensor_copy(ad1[:], psA[:])

        stop = os.environ.get("GAT_STOP", "full")

        # ---- layer 1: stream x[src] cells, matmul h1+a_s1, softmax-agg ----
        def tail1(w, aggn):
            t = spool.tile([128, 128], F32, tag="t1t")
            nc.vector.tensor_add(t[:], aggn[:], b1t_t[:])
            mn = spool.tile([128, 128], F32, tag="t1m")
            nc.vector.tensor_scalar_min(mn[:], t[:], 0.0)
            ex = spool.tile([128, 128], F32, tag="t1e")
            nc.scalar.activation(ex[:], mn[:], mybir.ActivationFunctionType.Exp)
            y = spool.tile([128, 128], F32, tag="t1x")
            nc.vector.scalar_tensor_tensor(y[:], t[:], 0.0, ex[:],
                                           mybir.AluOpType.max, mybir.AluOpType.add)
            nc.vector.tensor_scalar_sub(y[:], y[:], 1.0)
            yt_ps = ptpool.tile([128, 128], F32, tag="tr")
            nc.tensor.transpose(yt_ps[:], y[:], ident_t[:])
            yt = spool.tile([128, 128], F32, tag="t1xt")
            nc.scalar.copy(yt[:], yt_ps[:])
            ps = ptpool.tile([128, 136], F32, tag="psb")
            nc.tensor.matmul(ps[:], yt[:], rhs2_t[:], start=True, stop=True)
            rowt = spool.tile([128, 128], F32, tag="rowt")
            nc.scalar.copy(rowt[:].bitcast(F16)[:, 0:128], ps[:, 0:128])
            nc.vector.tensor_copy(rowt[:].bitcast(F16)[:, 128:132], ps[:, 128:132])
            nc.vector.memset(rowt[:, 66:128], 0.0)
            nc.vector.tensor_copy(ad2[:, 4 * w:4 * w + 4], ps[:, 132:136])
            nc.sync.dma_start(
                AGIN[w * 128:(w + 1) * 128, :].rearrange("(a p) r -> p a r", p=128),
                rowt[:].bitcast(F16).rearrange("p (a r) -> p a r", a=1))

        if stop != "a":
            # stream pieces: piece p covers chunks [p*PIECE, (p+1)*PIECE)
            piece_tiles = {}
            pend1 = None

            def get_piece(p):
                if p in piece_tiles:
                    return piece_tiles[p]
                k = min(PIECE, tot - p * PIECE)
                pt = strpool.tile([128, 128 * PIECE], F16, tag="xep")
                nc.sync.dma_start(pt[:, 0:128 * k],
                                  xeT[:, p * PIECE * 128:(p * PIECE + k) * 128])
                piece_tiles[p] = pt
                return pt

            nbatch = 0
            for w in range(WPC):
                d = int(d_w[w])
                cw = int(cs[w])
                xg = gpool.tile([128, d, L1W], F16, tag="xg1")
                # h1 + a_s1 per slot-column: 6 chunks per 2-bank PSUM tile
                # (3 chunks per bank; a matmul output cannot cross a bank)
                for j0 in range(0, d, 6):
                    bn = min(6, d - j0)
                    ps = pspool.tile([128, 1024], F32, tag="mm")
                    for k in range(bn):
                        g = cw + j0 + k
                        pt = get_piece(g // PIECE)
                        off = (g % PIECE) * 128
                        po = 512 * (k // 3) + 132 * (k % 3)
                        nc.tensor.matmul(ps[:, po:po + 132],
                                         pt[:, off:off + 128], rhs1_t[:],
                                         start=True, stop=True)
                    pv = ps[:].rearrange("p (b x) -> p b x", b=2)[:, :, 0:396]
                    pv = pv.rearrange("p b (k c) -> p b k c", c=132)
                    if bn == 6:
                        nc.scalar.copy(
                            xg[:, j0:j0 + 6, :].rearrange("p (b k) c -> p b k c", b=2),
                            pv)
                    else:
                        b0 = min(bn, 3)
                        nc.scalar.copy(xg[:, j0:j0 + b0, :], pv[:, 0, 0:b0, :])
                        if bn > 3:
                            nc.scalar.copy(xg[:, j0 + 3:j0 + bn, :],
                                           pv[:, 1, 0:bn - 3, :])
                ew = _logits_stage(nc, pools, w, d, xg[:, :, 128:132], ad1)
                if pend1 is not None:
                    _agg_stage(nc, pools, *pend1, den1, tail1)
                pend1 = (w, d, xg, ew)
            if pend1 is not None:
                _agg_stage(nc, pools, *pend1, den1, tail1)

        if stop in ("a", "l1"):
            nc.vector.memset(fin[:], 0.0)
        elif sim:
            nc.sync.dma_start(T2[0:NPAD, :], AGIN[:])
        else:
            nc.gpsimd.collective_compute(
                "AllGather", mybir.AluOpType.bypass,
                replica_groups=[list(range(NCORES))],
                ins=[AGIN[:].opt()], outs=[T2[:].opt()])
        if stop == "full":
            # pad row referenced by all pad cells: zero h2, a_s2 = NEGC
            nc.sync.dma_start(T2[PADROW:PADROW + 1, :], padrow_h[:])

        # ---- layer 2: dma_gather rows from T2, softmax-agg, project ----
        def tail2(w, aggn):
            at_ps = ptpool.tile([128, 128], F32, tag="tr")
            nc.tensor.transpose(at_ps[:], aggn[:], ident_t[:])
            at = spool.tile([128, 128], F32, tag="t2at")
            nc.scalar.copy(at[:], at_ps[:])
            ps8 = ptpool.tile([128, 136], F32, tag="psb")
            nc.tensor.matmul(ps8[:, 0:8], at[:], wout_t[:], start=True, stop=True)
            nc.vector.tensor_add(fin[:, OUTF * w:OUTF * (w + 1)], ps8[:, 0:8], bft_t[:])

        if stop == "ag":
            nc.vector.memset(fin[:], 0.0)
        if stop == "full":
            pend2 = None
            for w in range(WPC):
                d = int(d_w[w])
                cw = int(cs[w])
                xg = gpool.tile([128, d, ROWW], F16, tag="xg2")
                for q, (q0, q1) in enumerate(_qsegs(d)):
                    nq = 128 * (q1 - q0)
                    nc.gpsimd.dma_gather(xg[:, q0:q1, :], T2[BASE:, :],
                                         i2_t[:, 8 * (cw + q0):8 * (cw + q1)],
                                         nq, nq, ROWW, single_packet=False,
                                         queue_num=q)
                ew = _logits_stage(nc, pools, w, d, xg[:, :, 128:132], ad2, leaky_act=True)
                if pend2 is not None:
                    _agg_stage(nc, pools, *pend2, den2, tail2)
                pend2 = (w, d, xg, ew)
            if pend2 is not None:
                _agg_stage(nc, pools, *pend2, den2, tail2)

        nc.sync.dma_start(outy[:], fin[:])

    nc.compile()
    return nc


# ----------------------------------------------------------------------------
# entry point
# ----------------------------------------------------------------------------

def kernel(x, edge_index, W1, att_src1, att_dst1, b1, W2, att_src2, att_dst2,
           b2, Wout, bout):
    global LAST_RESULT, LAST_NC, LAST_IN_MAPS
    x = np.asarray(x, np.float32)
    edge_index = np.asarray(edge_index)

    ck = hash(edge_index.tobytes())
    if ck not in _CACHE:
        meta, perms = _host_prep(edge_index)
        nc = _build_program(meta)
        _CACHE.clear()
        _CACHE[ck] = (meta, perms, nc)
    meta, perms, nc = _CACHE[ck]
    tot = meta["tot"]

    rhs1, wdx1, rhs2, woutd, b1t, bft, cstm, v16 = _fold_weights(
        np.asarray(W1, np.float32), np.asarray(att_src1, np.float32),
        np.asarray(att_dst1, np.float32), np.asarray(b1, np.float32),
        np.asarray(W2, np.float32), np.asarray(att_src2, np.float32),
        np.asarray(att_dst2, np.float32), np.asarray(b2, np.float32),
        np.asarray(Wout, np.float32), np.asarray(bout, np.float32))

    ident = np.eye(128, dtype=np.float32)
    x16 = x.astype(np.float16)
    padrow = np.zeros((1, ROWW), np.float16)
    padrow[0, 128:132] = NEGC

    in_maps = []
    for c in range(NCORES):
        sc = meta["src_cell"][c]
        xeT = np.tile(v16, (tot * 128, 1))
        real = sc >= 0
        xeT[real] = x16[sc[real]]
        xs = np.zeros((128, NPAD), np.float16)
        xs[:, :NLOC] = x16[c * NLOC + perms[c]].T
        in_maps.append({
            "xeT": np.ascontiguousarray(xeT.T), "xs": xs, "rhs1": rhs1,
            "wdx1": wdx1, "rhs2": rhs2, "woutd": woutd, "b1t": b1t, "bft": bft,
            "cstm": cstm, "ident": ident, "padrow": padrow,
            "i2": np.ascontiguousarray(meta["idxw"][c]),
        })

    trace = bool(int(os.environ.get("GAT_TRACE", "0")))
    res = run_bass_kernel_spmd(nc, in_maps, core_ids=list(range(NCORES)),
                               trace=trace)
    LAST_RESULT = res
    LAST_NC, LAST_IN_MAPS = nc, in_maps

    out = np.empty((N, OUTF), np.float32)
    for c in range(NCORES):
        oy = res.results[c]["outy"].reshape(128, WPC, OUTF)
        oy = oy.transpose(1, 0, 2).reshape(NPAD, OUTF)
        out[c * NLOC + perms[c]] = oy[:NLOC]
    return out
